# revision 7
# baseline (speedup 1.0000x reference)
"""Bitnet-style GQA attention block on 8 trn2 NeuronCores.

Sharding: DP2 (batch) x TP4 (heads). Each core handles one batch element and
8 q-heads / 2 kv-heads, computing its slice of q/k/v proj, attention, and
o-proj. The o-proj is further split into 4 per-head-pair partials (one per
t = head-pair) shipped separately and summed on the host, so o-proj work
spreads evenly over all 16 attention chunks instead of piling up at the end.

Device-side layout is feature-major: activations live as [channels, tokens].
All matmuls are bf16 with fp32 PSUM accumulation. Softmax is unnormalized
(|scores| <= ~5), with the denominator via an all-ones column appended to V.

Pipeline structure (per core):
 - x streams in as 8 token-group descriptors (256 tokens each) on 2 queues;
   weights stream on a third queue. K-proj and V-proj(+transpose to
   token-major) consume each group as it lands, inside a nested PSUM scope.
 - 16 attention chunks (t=head-pair 0..3, qb=512-token block 0..3): scores
   (full-128-contraction via zero-padded Q halves), exp on the scalar engine
   (the pacing engine at ~1.1us/tile), PV with fused denominator, then
   normalize -> per-t A transpose -> per-t o-proj partial -> bf16 DMA out.
   The transpose/o-proj of chunk n and one just-in-time Q-proj block are
   injected into chunk n+1's kt loop to fill the scalar engine's slack.
"""

import numpy as np
import ml_dtypes
from contextlib import ExitStack

import concourse.bass as bass
import concourse.tile as tile
from concourse import bacc, mybir
from concourse.bass_utils import run_bass_kernel_spmd
from concourse.masks import make_identity

B, S, H = 2, 2048, 2048
N_HEADS, N_KV, HEAD_DIM = 32, 8, 64
N_CORES = 8
TP = 4                   # head-parallel degree per batch
QH = N_HEADS // TP       # 8 q-heads per core
KVH = N_KV // TP         # 2 kv heads per core
QCH = QH * HEAD_DIM      # 512
KCH = KVH * HEAD_DIM     # 128
ST = S // 128            # 16 token tiles
HK = H // 128            # 16 hidden-dim chunks
QB = 4                   # 512-wide q/token column blocks
NG = 8                   # x token groups
GT = S // NG             # 256 tokens per group
HEAD_ORDER = [0, 4, 1, 5, 2, 6, 3, 7]  # slot j -> local q-head index

F32 = mybir.dt.float32
BF16 = mybir.dt.bfloat16
BF16_NP = ml_dtypes.bfloat16

_CACHED_NC = None


def _build_nc():
    nc = bacc.Bacc("TRN2", target_bir_lowering=False, debug=False,
                   num_devices=N_CORES)

    xT = nc.dram_tensor("xT", [H, S], BF16, kind="ExternalInput").ap()
    wqT = nc.dram_tensor("wqT", [H, QCH], BF16, kind="ExternalInput").ap()
    wkT = nc.dram_tensor("wkT", [H, KCH], BF16, kind="ExternalInput").ap()
    wvT = nc.dram_tensor("wvT", [H, KCH], BF16, kind="ExternalInput").ap()
    woT = nc.dram_tensor("woT", [QCH, H], BF16, kind="ExternalInput").ap()
    # 4 per-head-pair o-proj partials, stacked: rows t*H..(t+1)*H
    outT = nc.dram_tensor("outT", [TP * H, S], BF16, kind="ExternalOutput").ap()

    with tile.TileContext(nc) as tc, ExitStack() as ctx:
        # ---- SBUF pools (whole kernel) ----
        xp = ctx.enter_context(tc.tile_pool(name="xp", bufs=NG))
        wqp = ctx.enter_context(tc.tile_pool(name="wqp", bufs=TP))
        wkvp = ctx.enter_context(tc.tile_pool(name="wkvp", bufs=2))
        wop = ctx.enter_context(tc.tile_pool(name="wop", bufs=1))
        ktp = ctx.enter_context(tc.tile_pool(name="ktp", bufs=1))
        vp = ctx.enter_context(tc.tile_pool(name="vp", bufs=ST))
        qtp = ctx.enter_context(tc.tile_pool(name="qtp", bufs=4))
        pexp = ctx.enter_context(tc.tile_pool(name="pexp", bufs=20))
        aqp = ctx.enter_context(tc.tile_pool(name="aqp", bufs=8))
        atp = ctx.enter_context(tc.tile_pool(name="atp", bufs=2))
        smp = ctx.enter_context(tc.tile_pool(name="smp", bufs=4))
        stg = ctx.enter_context(tc.tile_pool(name="stg", bufs=4))
        rcp = ctx.enter_context(tc.tile_pool(name="rcp", bufs=16))
        cst = ctx.enter_context(tc.tile_pool(name="cst", bufs=1))

        ident = cst.tile([128, 128], BF16, tag="ident")
        make_identity(nc, ident[:])

        # ---- SBUF tiles + merged input DMA ----
        # x token groups: xg[g][p, hk*GT + c] = x[hk*128+p, g*GT+c]
        xg = [xp.tile([128, HK * GT], BF16, tag="xg", name=f"xg{g}")
              for g in range(NG)]
        wk_sb = wkvp.tile([128, HK * 128], BF16, tag="wk")
        wv_sb = wkvp.tile([128, HK * 128], BF16, tag="wv")
        wq_sb = [wqp.tile([128, HK * 128], BF16, tag="wq", name=f"wq{t}")
                 for t in range(TP)]
        wo_sb = wop.tile([128, TP * H], BF16, tag="wo")

        def merged_in(dst_tile, src_ap, inner):
            return (dst_tile[:].rearrange("p (k c) -> p k c", c=inner),
                    src_ap.rearrange("(k p) c -> p k c", p=128))

        # weights queue (gpsimd): wk, wq_t0, wv, wq_t1..3, wo
        d, s_ = merged_in(wk_sb, wkT, 128)
        nc.gpsimd.dma_start(d, s_)
        d, s_ = merged_in(wq_sb[0], wqT[:, 0:128], 128)
        nc.gpsimd.dma_start(d, s_)
        d, s_ = merged_in(wv_sb, wvT, 128)
        nc.gpsimd.dma_start(d, s_)
        # x groups: evens on sync, odds on vector
        for g in range(NG):
            d, s_ = (xg[g][:].rearrange("p (k c) -> p k c", c=GT),
                     xT[:, g * GT:(g + 1) * GT].rearrange(
                         "(k p) c -> p k c", p=128))
            (nc.sync if g % 2 == 0 else nc.scalar).dma_start(d, s_)
        for t in range(1, TP):
            d, s_ = merged_in(wq_sb[t], wqT[:, t * 128:(t + 1) * 128], 128)
            nc.gpsimd.dma_start(d, s_)
        d, s_ = merged_in(wo_sb, woT, H)
        nc.gpsimd.dma_start(d, s_)

        kt_sb = ktp.tile([128, S], BF16, tag="kt")
        vones = [vp.tile([128, 130], BF16, tag="vones", name=f"vt{st}")
                 for st in range(ST)]
        for st in range(ST):
            nc.gpsimd.memset(vones[st][:, 64:65], 1.0)
            nc.gpsimd.memset(vones[st][:, 129:130], 1.0)

        qpad_of = {}

        def emit_qpad_alloc(t):
            qpad = []
            for h in range(2):
                qp = qtp.tile([128, S], BF16, tag="qt", name=f"qp{t}_{h}")
                lo = (1 - h) * 64  # zero half
                nc.vector.memset(qp[lo:lo + 64, :], 0.0)
                qpad.append(qp)
            qpad_of[t] = qpad

        def emit_qproj_block(t, sb, pool):
            # one 512-token block of pair t's Q projection
            qpad = qpad_of[t]
            pq = pool.tile([128, 512], F32, tag="scr", name="pq")
            for hk in range(HK):
                for j, g in enumerate((2 * sb, 2 * sb + 1)):
                    # pq is one PSUM bank shared by both column halves:
                    # start=True clears the whole bank, so only the first
                    # half's hk=0 matmul sets it; the second half overwrites
                    # on first touch via per-element has_written bits.
                    nc.tensor.matmul(
                        pq[:, j * GT:(j + 1) * GT],
                        wq_sb[t][:, hk * 128:(hk + 1) * 128],
                        xg[g][:, hk * GT:(hk + 1) * GT],
                        start=(hk == 0 and j == 0), stop=(hk == HK - 1),
                        skip_group_check=True)
            cols = slice(sb * 512, (sb + 1) * 512)
            nc.vector.tensor_copy(qpad[0][0:64, cols], pq[0:64, :])
            nc.vector.tensor_copy(qpad[1][64:128, cols], pq[64:128, :])

        # ---- load phase: K/V proj per x-group as it lands ----
        with tc.tile_pool(name="kb", bufs=2, space="PSUM") as kb, \
             tc.tile_pool(name="vb", bufs=2, space="PSUM") as vb, \
             tc.tile_pool(name="spL", bufs=2, space="PSUM") as spL:
            for g in range(NG):
                pk = kb.tile([128, GT], F32, tag="pk")
                for hk in range(HK):
                    nc.tensor.matmul(pk[:], wk_sb[:, hk * 128:(hk + 1) * 128],
                                     xg[g][:, hk * GT:(hk + 1) * GT],
                                     start=(hk == 0), stop=(hk == HK - 1))
                nc.vector.tensor_copy(kt_sb[:, g * GT:(g + 1) * GT], pk[:])
                pv = vb.tile([128, GT], F32, tag="pv")
                for hk in range(HK):
                    nc.tensor.matmul(pv[:], wv_sb[:, hk * 128:(hk + 1) * 128],
                                     xg[g][:, hk * GT:(hk + 1) * GT],
                                     start=(hk == 0), stop=(hk == HK - 1))
                vtsb = stg.tile([128, GT], BF16, tag="vtsb")
                nc.vector.tensor_copy(vtsb[:], pv[:])
                for bb in range(2):
                    st = 2 * g + bb
                    pt = spL.tile([128, 128], BF16, tag="spL", name="ptv")
                    nc.tensor.transpose(pt[:], vtsb[:, bb * 128:(bb + 1) * 128],
                                        ident[:])
                    nc.vector.tensor_copy(vones[st][:, 0:64], pt[:, 0:64])
                    nc.vector.tensor_copy(vones[st][:, 65:129], pt[:, 64:128])
            emit_qpad_alloc(0)
            emit_qproj_block(0, 0, spL)

        # ---- body: 16 attention chunks ----
        big = ctx.enter_context(tc.tile_pool(name="big", bufs=2, space="PSUM"))
        pap = ctx.enter_context(tc.tile_pool(name="pap", bufs=2, space="PSUM"))
        scr = ctx.enter_context(tc.tile_pool(name="scr", bufs=2, space="PSUM"))

        # deferred transpose + o-proj partial jobs of the previous chunk
        def make_tp_job(t, qb, aq):
            def job():
                at_t = atp.tile([128, 512], BF16, tag="at", name=f"at{t}_{qb}")
                for sq in range(4):
                    pt = scr.tile([128, 128], BF16, tag="scr", name="ptr")
                    nc.tensor.transpose(pt[:], aq[sq][:], ident[:])
                    nc.vector.tensor_copy(at_t[:, sq * 128:(sq + 1) * 128],
                                          pt[:])
                at_of[(t, qb)] = at_t
            return job

        def make_oproj_job(t, qb, og):
            def job():
                at_t = at_of[(t, qb)]
                so = smp.tile([128, 4 * 512], BF16, tag="so")
                for j in range(4):
                    ot = og * 4 + j
                    po = scr.tile([128, 512], F32, tag="scr", name="po")
                    nc.tensor.matmul(
                        po[:], wo_sb[:, t * H + ot * 128: t * H + (ot + 1) * 128],
                        at_t[:], start=True, stop=True)
                    nc.vector.tensor_copy(so[:, j * 512:(j + 1) * 512], po[:])
                nc.sync.dma_start(
                    outT[t * H + og * 512: t * H + (og + 1) * 512,
                         qb * 512:(qb + 1) * 512].rearrange(
                             "(k p) c -> p k c", p=128),
                    so[:].rearrange("p (k c) -> p k c", c=512))
            return job

        at_of = {}
        chunks = [(t, qb) for t in range(4) for qb in range(4)]
        # just-in-time Q-proj blocks: chunk c hosts block c+2 (t=n//4, sb=n%4)
        pending = []  # deferred jobs from previous chunk

        # chunk c hosts Q-proj block c+2 (and chunk 0 additionally block 1):
        # block m = (t=m//4, sb=m%4) completes during chunk m-2 <= m-1.
        qjobs = {0: [1, 2]}
        for c in range(1, 14):
            qjobs[c] = [c + 2]

        for c, (t, qb) in enumerate(chunks):
            inject = list(pending)
            pending = []
            for m in qjobs.get(c, []):
                tq, sbq = m // 4, m % 4
                if sbq == 0:
                    inject.append(lambda tq=tq: emit_qpad_alloc(tq))
                inject.append(
                    lambda tq=tq, sbq=sbq: emit_qproj_block(tq, sbq, scr))

            qpad = qpad_of[t]
            qcols = slice(qb * 512, (qb + 1) * 512)
            ptile = [None] * ST
            pa = [None, None]

            def emit_pv(kt):
                for h in range(2):
                    for qt in range(4):
                        nc.tensor.matmul(
                            pa[h][:, qt * 65:qt * 65 + 65],
                            ptile[kt][:, h * 512 + qt * 128:
                                      h * 512 + (qt + 1) * 128],
                            vones[kt][:, h * 65:h * 65 + 65],
                            start=(kt == 0 and qt == 0),
                            stop=(kt == ST - 1 and qt == 3),
                            skip_group_check=True)

            for kt in range(ST):
                ps2 = big.tile([128, 1024], F32, tag="big")
                for h in range(2):
                    nc.tensor.matmul(
                        ps2[:, h * 512:(h + 1) * 512],
                        kt_sb[:, kt * 128:(kt + 1) * 128],
                        qpad[h][:, qcols],
                        start=True, stop=True)
                pe = pexp.tile([128, 1024], BF16, tag="pexp")
                nc.scalar.activation(pe[:], ps2[:],
                                     mybir.ActivationFunctionType.Exp,
                                     scale=0.125)
                ptile[kt] = pe
                if kt == 1:
                    pa[0] = pap.tile([128, 260], F32, tag="pa", name="pa0")
                    pa[1] = pap.tile([128, 260], F32, tag="pa", name="pa1")
                if kt >= 2:
                    emit_pv(kt - 2)
                if kt >= 2 and kt % 2 == 0 and inject:
                    inject.pop(0)()
            emit_pv(ST - 2)
            emit_pv(ST - 1)
            for f in inject:
                f()

            # normalize into q-major per-t channel tiles
            aq = [aqp.tile([128, 128], BF16, tag="aq", name=f"aq{sq}")
                  for sq in range(4)]
            for h in range(2):
                for qt in range(4):
                    rc = rcp.tile([128, 1], F32, tag="rc")
                    nc.vector.reciprocal(rc[:],
                                         pa[h][:, qt * 65 + 64:qt * 65 + 65])
                    nc.vector.tensor_scalar_mul(
                        aq[qt][:, h * 64:(h + 1) * 64],
                        pa[h][:, qt * 65:qt * 65 + 64], rc[:])

            pending.append(make_tp_job(t, qb, aq))
            for og in range(4):
                pending.append(make_oproj_job(t, qb, og))

        for f in pending:
            f()

    nc.compile()
    return nc


def _get_nc():
    global _CACHED_NC
    if _CACHED_NC is None:
        _CACHED_NC = _build_nc()
    return _CACHED_NC


def _prep_core_inputs(hidden_states, Wq, Wk, Wv, Wo):
    """Host-side shard + transpose + bf16 cast. Returns list of 8 input dicts."""
    xT_b = []
    for b in range(B):
        xT_b.append(np.ascontiguousarray(hidden_states[b].T).astype(BF16_NP))
    in_maps = []
    for c in range(N_CORES):
        b, g = divmod(c, TP)
        wq_rows = np.concatenate([
            Wq[(g * QH + h) * HEAD_DIM:(g * QH + h + 1) * HEAD_DIM, :]
            for h in HEAD_ORDER], axis=0)            # [512, H]
        wo_cols = np.concatenate([
            Wo[:, (g * QH + h) * HEAD_DIM:(g * QH + h + 1) * HEAD_DIM]
            for h in HEAD_ORDER], axis=1)            # [H, 512]
        in_maps.append({
            "xT": xT_b[b],
            "wqT": np.ascontiguousarray(wq_rows.T).astype(BF16_NP),
            "wkT": np.ascontiguousarray(Wk[g * KCH:(g + 1) * KCH, :].T).astype(BF16_NP),
            "wvT": np.ascontiguousarray(Wv[g * KCH:(g + 1) * KCH, :].T).astype(BF16_NP),
            "woT": np.ascontiguousarray(wo_cols.T).astype(BF16_NP),
        })
    return in_maps


def _combine(results):
    out = np.empty((B, S, H), dtype=np.float32)
    for b in range(B):
        acc = None
        for g in range(TP):
            o = results[b * TP + g]["outT"].astype(np.float32)
            o = o.reshape(TP, H, S).sum(axis=0)
            acc = o if acc is None else acc + o
        out[b] = acc.T
    return out


def kernel(hidden_states, attention_mask, Wq, Wk, Wv, Wo):
    # attention_mask is all zeros for this problem spec; softmax is invariant
    # to the zero additive mask, so it is not shipped to the device.
    hidden_states = np.asarray(hidden_states)
    nc = _get_nc()
    in_maps = _prep_core_inputs(hidden_states, np.asarray(Wq), np.asarray(Wk),
                                np.asarray(Wv), np.asarray(Wo))
    res = run_bass_kernel_spmd(nc, in_maps, list(range(N_CORES)))
    return _combine(res.results)
